# revision 1
# baseline (speedup 1.0000x reference)
"""Trainium2 Bass kernel for masked cross-attention (nn_CausalAttention).

Reference computation (per batch):
    q  = x @ Wq                       # [128, 1024]
    kv = context @ Wkv; k, v = split  # [4096, 1024] each
    per head h (16 heads, dim 64):
        sim[i, j] = (q_h[i] . k_h[j]) * 0.125, masked to j % 128 == i
        out_h = softmax(sim) @ v_h
    y = concat_h(out) @ Wout + bout

The mask (j % 128) == i means query i attends exactly the 32 keys
j = i + 128*t.  KV-projection token-tile t lands in SBUF as
[128 tokens, 1024 feats] with token i on partition i, so the scores are
per-partition dot products (DVE elementwise mul + segmented reduce) and the
attention-weighted V sum is a per-partition broadcast-mul accumulate.  The
dense [128, 4096] similarity matrix is never formed.

Sharding: data-parallel over batch, 2 batches per core, no collectives.
Host pre-transposes x and context to feat-major so every matmul operand has
the contraction dim on partitions with no on-chip transposes.  Matmuls run
in float32r (single-pass fp22).
"""

import numpy as np
from contextlib import ExitStack

import concourse.bass as bass
import concourse.tile as tile
from concourse import bacc, mybir
from concourse.bass_utils import run_bass_kernel_spmd
from concourse.masks import make_identity

FP = mybir.dt.float32
FPR = mybir.dt.float32r
BF16 = mybir.dt.bfloat16
MMDT = FPR  # matmul operand dtype (FPR or BF16), set by build_kernel
ABLATE_ATTN = False  # timing diagnostic: drop DVE attention ops
SCORE_BF16 = False   # q/k tiles in bf16 for 2x DVE score muls
AV_PSUM = False      # accumulate weighted V in PSUM via identity matmuls
STRIP_SYNC = False   # ctx strips on HWDGE (sync) instead of SWDGE (gpsimd)
KVT_BUFS = 2
CTXS_BUFS = 12
PSUM_BUFS = 4
TR_SHARE = False
PROD_BUFS = 3
AX = mybir.AxisListType
ALU = mybir.AluOpType
ACTF = mybir.ActivationFunctionType

B, NQ, NKV, DIM, H, DH = 16, 128, 4096, 1024, 16, 64
INNER = H * DH  # 1024
SCALE = DH ** -0.5  # 0.125
N_CORES = 8
BPC = B // N_CORES  # batches per core
KT = DIM // 128     # 8 contraction chunks
NT = INNER // 512   # 2 output-feature chunks of 512
TT = NKV // NQ      # 32 key tiles per query row
TG = 4              # t-tiles per ctx strip load ([128, 512] strips)


def _body(tc, xT, ctxT, wq, wkv, wout, bout, y, bpc=BPC, pfx=""):
    nc = tc.nc
    BPC = bpc
    mmcast = (lambda ap: ap.bitcast(FPR)) if MMDT is FPR else (lambda ap: ap)
    with ExitStack() as ctx:
        ep = ctx.enter_context

        wkv_p = ep(tc.tile_pool(name=pfx + "wkv", bufs=2 * KT * NT))      # 64KB/part
        wqo_p = ep(tc.tile_pool(name=pfx + "wqo", bufs=KT * NT))          # 32KB/part
        ctxs_p = ep(tc.tile_pool(name=pfx + "ctxs", bufs=CTXS_BUFS))
        xt_p = ep(tc.tile_pool(name=pfx + "xt", bufs=KT))
        q_p = ep(tc.tile_pool(name=pfx + "q", bufs=BPC))
        kvt_p = ep(tc.tile_pool(name=pfx + "kvt", bufs=KVT_BUFS))
        prod_p = ep(tc.tile_pool(name=pfx + "prod", bufs=PROD_BUFS))
        acc_p = ep(tc.tile_pool(name=pfx + "acc", bufs=2))
        sim_p = ep(tc.tile_pool(name=pfx + "sim", bufs=2))
        exp_p = ep(tc.tile_pool(name=pfx + "exp", bufs=2))
        stat_p = ep(tc.tile_pool(name=pfx + "stat", bufs=8))
        ot_p = ep(tc.tile_pool(name=pfx + "ot", bufs=KT))
        yb_p = ep(tc.tile_pool(name=pfx + "yb", bufs=1))
        outn_p = ep(tc.tile_pool(name=pfx + "outn", bufs=2))
        const_p = ep(tc.tile_pool(name=pfx + "const", bufs=1))
        psum_p = ep(tc.tile_pool(name=pfx + "psum", bufs=PSUM_BUFS, space="PSUM"))
        psum_tr_p = (None if TR_SHARE else
                     ep(tc.tile_pool(name=pfx + "psumtr", bufs=2, space="PSUM")))
        psum_av_p = (ep(tc.tile_pool(name=pfx + "psumav", bufs=2, space="PSUM"))
                     if AV_PSUM else None)

        # ---- weights: Wq + x first (critical path to the first matmul) ----
        wq_t = {}
        for k in range(KT):
            for n in range(NT):
                t = wqo_p.tile([128, 512], MMDT, tag="wqo")
                nc.sync.dma_start(
                    t[:], mmcast(wq[k * 128:(k + 1) * 128,
                                    n * 512:(n + 1) * 512]))
                wq_t[k, n] = t

        # ---- Q projection (both batches), scores scale folded into evac ----
        q_sb = []
        for b in range(BPC):
            xt = []
            for k in range(KT):
                t = xt_p.tile([128, 128], MMDT, tag="xt")
                nc.gpsimd.dma_start(
                    t[:], mmcast(xT[b, k * 128:(k + 1) * 128, :]))
                xt.append(t)
            q = q_p.tile([128, INNER], BF16 if SCORE_BF16 else FP, tag="q")
            for n in range(NT):
                ps = psum_p.tile([128, 512], FP, tag="ps")
                for k in range(KT):
                    nc.tensor.matmul(
                        ps[:], xt[k][:], wq_t[k, n][:],
                        start=(k == 0), stop=(k == KT - 1))
                nc.scalar.activation(
                    q[:, n * 512:(n + 1) * 512], ps[:], ACTF.Copy, scale=SCALE)
            q_sb.append(q)

        wk_t, wv_t, wout_t = {}, {}, {}

        def load_w(dst, src, k, n, coff, pool, tag):
            t = pool.tile([128, 512], MMDT, tag=tag)
            nc.sync.dma_start(
                t[:], mmcast(src[k * 128:(k + 1) * 128,
                                 coff + n * 512:coff + (n + 1) * 512]))
            dst[k, n] = t

        for k in range(KT):
            for n in range(NT):
                load_w(wk_t, wkv, k, n, 0, wkv_p, "wkv")
        for k in range(KT):
            for n in range(NT):
                load_w(wv_t, wkv, k, n, INNER, wkv_p, "wkv")
        # Wout reuses the Wq pool slots once q-projection has consumed them.
        for k in range(KT):
            for n in range(NT):
                load_w(wout_t, wout, k, n, 0, wqo_p, "wqo")

        ident = const_p.tile([128, 128], FP, tag="ident")
        make_identity(nc, ident[:])
        identr = const_p.tile([128, 128], FPR, tag="identr")
        nc.scalar.activation(identr[:], ident[:], ACTF.Copy)
        bout_sb = const_p.tile([128, INNER], FP, tag="bout")
        nc.sync.dma_start(bout_sb[:], bout[:, :])

        def kv_tile(b, t_idx, strips, w_t, dt=FP, tag="kvt", pool=None):
            """Project ctx token-tile t through Wk/Wv half -> SBUF [128, 1024]."""
            tj = t_idx % TG
            kv = (pool or kvt_p).tile([128, INNER], dt, tag=tag)
            for n in range(NT):
                ps = psum_p.tile([128, 512], FP, tag="ps")
                for k in range(KT):
                    lhsT = strips[k][:, tj * 128:(tj + 1) * 128]
                    nc.tensor.matmul(
                        ps[:], lhsT, w_t[k, n][:],
                        start=(k == 0), stop=(k == KT - 1))
                nc.scalar.activation(
                    kv[:, n * 512:(n + 1) * 512], ps[:], ACTF.Copy)
            return kv

        def load_strips(b, tg):
            strips = []
            for k in range(KT):
                s = ctxs_p.tile([128, 128 * TG], MMDT, tag="ctxs")
                eng = nc.sync if STRIP_SYNC else nc.gpsimd
                eng.dma_start(
                    s[:], mmcast(ctxT[b, k * 128:(k + 1) * 128,
                                      tg * 128 * TG:(tg + 1) * 128 * TG]))
                strips.append(s)
            return strips

        def pass1(b):
            """K tiles -> sparse scores -> softmax; returns (ex3, rec)."""
            sink = []
            sim = sim_p.tile([128, H * TT], FP, tag="sim")
            sim3 = sim[:].rearrange("p (h t) -> p h t", h=H)
            for tg in range(TT // TG):
                strips = load_strips(b, tg)
                for tj in range(TG):
                    t_idx = tg * TG + tj
                    kt = kv_tile(b, t_idx, strips, wk_t,
                                 dt=BF16 if SCORE_BF16 else FP)
                    if ABLATE_ATTN:
                        sink.append(kt)
                        continue
                    pr = prod_p.tile([128, INNER],
                                     BF16 if SCORE_BF16 else FP, tag="prod")
                    nc.vector.tensor_tensor(
                        pr[:], q_sb[b][:], kt[:], op=ALU.mult)
                    nc.vector.reduce_sum(
                        sim3[:, :, t_idx:t_idx + 1],
                        pr[:].rearrange("p (h d) -> p h d", h=H), axis=AX.X)

            if ABLATE_ATTN:
                return None, None
            rmax = stat_p.tile([128, H], FP, tag="rmax")
            nc.vector.reduce_max(rmax[:], sim3, axis=AX.X)
            shift = sim_p.tile([128, H * TT], FP, tag="shift")
            nc.vector.tensor_tensor(
                shift[:].rearrange("p (h t) -> p h t", h=H), sim3,
                rmax[:, :, None].broadcast_to([128, H, TT]), op=ALU.subtract)
            ex = exp_p.tile([128, H * TT], FP, tag="exp")
            nc.scalar.activation(ex[:], shift[:], ACTF.Exp)
            ex3 = ex[:].rearrange("p (h t) -> p h t", h=H)
            den = stat_p.tile([128, H], FP, tag="den")
            nc.vector.reduce_sum(den[:], ex3, axis=AX.X)
            rec = stat_p.tile([128, H], FP, tag="rec")
            nc.vector.reciprocal(rec[:], den[:])
            return ex3, rec

        def pass2(b, ex3, rec):
            """V tiles -> normalized attention output [128, (h, d)]."""
            if AV_PSUM and not ABLATE_ATTN:
                return pass2_psum(b, ex3, rec)
            acc = None
            for tg in range(TT // TG):
                strips = load_strips(b, tg)
                for tj in range(TG):
                    t_idx = tg * TG + tj
                    vt = kv_tile(b, t_idx, strips, wv_t)
                    if ABLATE_ATTN:
                        continue
                    ebc = ex3[:, :, t_idx:t_idx + 1].broadcast_to([128, H, DH])
                    vt3 = vt[:].rearrange("p (h d) -> p h d", h=H)
                    if acc is None:
                        acc = acc_p.tile([128, INNER], FP, tag="acc")
                        nc.vector.tensor_tensor(
                            acc[:].rearrange("p (h d) -> p h d", h=H),
                            vt3, ebc, op=ALU.mult)
                    else:
                        wv = prod_p.tile([128, INNER], FP, tag="prod")
                        nc.vector.tensor_tensor(
                            wv[:].rearrange("p (h d) -> p h d", h=H),
                            vt3, ebc, op=ALU.mult)
                        acc2 = acc_p.tile([128, INNER], FP, tag="acc")
                        nc.vector.tensor_tensor(
                            acc2[:], acc[:], wv[:], op=ALU.add)
                        acc = acc2

            if ABLATE_ATTN:
                return bout_sb
            out_n = outn_p.tile([128, INNER], FP, tag="outn")
            nc.vector.tensor_tensor(
                out_n[:].rearrange("p (h d) -> p h d", h=H),
                acc[:].rearrange("p (h d) -> p h d", h=H),
                rec[:, :, None].broadcast_to([128, H, DH]), op=ALU.mult)
            return out_n

        def pass2_psum(b, ex3, rec):
            """V pass with the weighted-V sum accumulated in PSUM by PE.

            The identity matmul for tile t is emitted one t later so the
            DVE multiply never stalls the PE stream.
            """
            ps_av = [psum_av_p.tile([128, 512], FP, tag="av", name=f"av{n}")
                     for n in range(NT)]
            wv_prev = None
            t_prev = -1

            def emit_identity_mm(wv, t_idx):
                for n in range(NT):
                    nc.tensor.matmul(
                        ps_av[n][:], identr[:],
                        wv[:, n * 512:(n + 1) * 512],
                        start=(t_idx == 0), stop=(t_idx == TT - 1),
                        skip_group_check=True)

            for tg in range(TT // TG):
                strips = load_strips(b, tg)
                for tj in range(TG):
                    t_idx = tg * TG + tj
                    vt = kv_tile(b, t_idx, strips, wv_t)
                    if wv_prev is not None:
                        emit_identity_mm(wv_prev, t_prev)
                    ebc = ex3[:, :, t_idx:t_idx + 1].broadcast_to([128, H, DH])
                    wv = prod_p.tile([128, INNER], FPR, tag="wv")
                    nc.vector.tensor_tensor(
                        wv[:].rearrange("p (h d) -> p h d", h=H),
                        vt[:].rearrange("p (h d) -> p h d", h=H), ebc,
                        op=ALU.mult)
                    wv_prev, t_prev = wv, t_idx
            emit_identity_mm(wv_prev, t_prev)

            out_n = outn_p.tile([128, INNER], FP, tag="outn")
            for n in range(NT):
                nc.vector.tensor_tensor(
                    out_n[:, n * 512:(n + 1) * 512]
                    .rearrange("p (h d) -> p h d", h=H // NT),
                    ps_av[n][:].rearrange("p (h d) -> p h d", h=H // NT),
                    rec[:, n * (H // NT):(n + 1) * (H // NT), None]
                    .broadcast_to([128, H // NT, DH]), op=ALU.mult)
            return out_n

        def outproj(b, out_n):
            """Transpose out_n on PE, then @ Wout + bout -> y[b]."""
            ot = []
            for k in range(KT):
                if TR_SHARE:
                    pst = psum_p.tile([128, 512], FP, tag="ps", name="pst")
                else:
                    pst = psum_tr_p.tile([128, 128], FP, tag="pst")
                nc.tensor.transpose(
                    pst[:, :128], out_n[:, k * 128:(k + 1) * 128], ident[:])
                o = ot_p.tile([128, 128], MMDT, tag="ot")
                nc.scalar.activation(o[:], pst[:, :128], ACTF.Copy)
                ot.append(o)
            yb = yb_p.tile([128, INNER], FP, tag="yb")
            for n in range(NT):
                ps = psum_p.tile([128, 512], FP, tag="ps")
                for k in range(KT):
                    nc.tensor.matmul(
                        ps[:], ot[k][:], wout_t[k, n][:],
                        start=(k == 0), stop=(k == KT - 1))
                nc.vector.tensor_tensor(
                    yb[:, n * 512:(n + 1) * 512], ps[:],
                    bout_sb[:, n * 512:(n + 1) * 512], op=ALU.add)
            nc.sync.dma_start(y[b], yb[:])

        # Software pipeline across batches: batch b's output projection is
        # emitted after batch b+1's pass 1 so the PE never waits on the
        # serial DVE attention chain (except at the very tail).
        pending = None  # (b, out_n)
        for b in range(BPC):
            ex3, rec = pass1(b)
            if pending is not None:
                outproj(*pending)
            out_n = pass2(b, ex3, rec)
            pending = (b, out_n)
        outproj(*pending)


def build_kernel(bpc=BPC, repeats=1, loop=0, mmdt="fpr", ablate_attn=False,
                 score_bf16=False, av_psum=False, tg=4, strip_sync=False,
                 kvt_bufs=2, ctxs_bufs=12, psum_bufs=4, tr_share=False,
                 prod_bufs=3):
    global MMDT, ABLATE_ATTN, SCORE_BF16, AV_PSUM, TG, STRIP_SYNC
    global KVT_BUFS, CTXS_BUFS, PSUM_BUFS, TR_SHARE, PROD_BUFS
    PSUM_BUFS = psum_bufs
    TR_SHARE = tr_share
    PROD_BUFS = prod_bufs
    MMDT = FPR if mmdt == "fpr" else BF16
    ABLATE_ATTN = ablate_attn
    SCORE_BF16 = score_bf16
    AV_PSUM = av_psum
    TG = tg
    STRIP_SYNC = strip_sync
    KVT_BUFS = kvt_bufs
    CTXS_BUFS = ctxs_bufs
    iodt = FP if MMDT is FPR else BF16
    nc = bacc.Bacc("TRN2", target_bir_lowering=False, debug=False)
    xT = nc.dram_tensor("xT", [bpc, DIM, NQ], iodt, kind="ExternalInput").ap()
    ctxT = nc.dram_tensor("ctxT", [bpc, DIM, NKV], iodt, kind="ExternalInput").ap()
    wq = nc.dram_tensor("wq", [DIM, INNER], iodt, kind="ExternalInput").ap()
    wkv = nc.dram_tensor("wkv", [DIM, 2 * INNER], iodt, kind="ExternalInput").ap()
    wout = nc.dram_tensor("wout", [INNER, DIM], iodt, kind="ExternalInput").ap()
    bout = nc.dram_tensor("bout", [128, DIM], FP, kind="ExternalInput").ap()
    y = nc.dram_tensor("y", [bpc, NQ, DIM], FP, kind="ExternalOutput").ap()

    with tile.TileContext(nc) as tc:
        if loop:
            with tc.For_i(0, loop, 1):
                _body(tc, xT, ctxT, wq, wkv, wout, bout, y, bpc=bpc)
        else:
            for r in range(repeats):
                _body(tc, xT, ctxT, wq, wkv, wout, bout, y, bpc=bpc,
                      pfx=f"r{r}_" if repeats > 1 else "")
    nc.compile()
    return nc


_NC_CACHE = {}


def make_in_maps(x, context, Wq, Wkv, Wout, bout):
    import ml_dtypes
    hdt = np.float32 if MMDT is FPR else ml_dtypes.bfloat16
    x = np.ascontiguousarray(x, dtype=np.float32)
    context = np.ascontiguousarray(context, dtype=np.float32)
    bout_rep = np.ascontiguousarray(
        np.broadcast_to(bout.astype(np.float32), (128, DIM)))
    w = {
        "wq": np.ascontiguousarray(Wq, dtype=hdt),
        "wkv": np.ascontiguousarray(Wkv, dtype=hdt),
        "wout": np.ascontiguousarray(Wout, dtype=hdt),
        "bout": bout_rep,
    }
    in_maps = []
    for c in range(N_CORES):
        sl = slice(c * BPC, (c + 1) * BPC)
        xT = np.ascontiguousarray(x[sl].transpose(0, 2, 1).astype(hdt))
        ctxT = np.ascontiguousarray(context[sl].transpose(0, 2, 1).astype(hdt))
        in_maps.append({"xT": xT, "ctxT": ctxT, **w})
    return in_maps


def kernel(x, context, Wq, Wkv, Wout, bout):
    if "nc" not in _NC_CACHE:
        _NC_CACHE["nc"] = build_kernel()
    nc = _NC_CACHE["nc"]
    in_maps = make_in_maps(x, context, Wq, Wkv, Wout, bout)
    res = run_bass_kernel_spmd(nc, in_maps, list(range(N_CORES)))
    out = np.concatenate([res.results[c]["y"] for c in range(N_CORES)], axis=0)
    return out.astype(np.float32)

